# revision 2
# baseline (speedup 1.0000x reference)
"""GCN encoder (3x gcn_conv) on 8 Trainium2 NeuronCores — sender-side design.

Edges are sharded by SOURCE node owner. Per core:
  tab:  h1 = x_own @ W1 (bf16 table in local DRAM; 49 block matmuls from xT —
        no transposes since x arrives feature-major).
  L1:   edges are packed into 1024-edge "units" (8 chunks of 128; the SWDGE
        ucode caps one dma_gather/dma_scatter_add at 1024 indices). Per unit:
        one dma_gather of h1[src] (int16 idxs), per chunk two PE matmuls into
        PSUM (eps = at@We_aug, += I@g), msg = relu(eps)*srw on DVE/ACT
        (alternating), then one dma_scatter_add of msg into the global
        partial table [50000,128] bf16. Units are 'lo' (dst<25000) or 'hi'
        (dst>=25000) so relative idxs fit int16. The host schedules each
        dst's edges into distinct units, so a scatter call never has
        duplicate rows; pad slots point at a row unused by that call with
        msg forced to 0 via srw=0.
  RS1:  ReduceScatter(add) partial1 -> this core's 6250 rows of agg1.
  H:    h = relu(agg1 + relu(h1+b1+root1)/deg); t2 = h @ [Wmu|Wls] (bf16).
  L2:   same edge pass against t2/we2aug -> partial2 -> RS2 -> + self2 -> out.
The GCN norm dinv[row]*dinv[col] is folded into the per-edge srw scale.
"""
import numpy as np

N_NODES = 50000
N_CORES = 8
SHARD = N_NODES // N_CORES          # 6250
HALF = N_NODES // 2                 # 25000 (int16 scatter windows)
P = 128
NB = (SHARD + P - 1) // P           # 49 own blocks
IN_F = 128
HID = 128
OUT_F = 64
UC = 8                              # chunks per unit (1024-idx SWDGE cap)
USZ = UC * P                        # 1024 edges per unit
ATB = 4                             # units per at-stream tile


def _bf16(x):
    import ml_dtypes
    return np.asarray(x, dtype=ml_dtypes.bfloat16)


def _assign_units(d_rel, U):
    """Assign edges (dst ids d_rel) to U units with no dst repeated within a
    unit: dsts ranked by in-degree descending; k-th edge of rank-i dst ->
    unit (i+k) % U. Per-unit load <= ceil-sum bound ~ E/U + max_indeg."""
    ne = d_rel.shape[0]
    if ne == 0:
        return np.zeros(0, np.int64)
    uniq, inv, counts = np.unique(d_rel, return_inverse=True, return_counts=True)
    rank = np.empty(len(uniq), np.int64)
    rank[np.argsort(-counts, kind="stable")] = np.arange(len(uniq))
    order = np.argsort(inv, kind="stable")
    kpos = np.empty(ne, np.int64)
    seg_start = np.concatenate([[0], np.cumsum(counts)])[:-1]
    kpos[order] = np.arange(ne) - seg_start[inv[order]]
    return (rank[inv] + kpos) % U


def _host_prep(x, edge_index, edge_attr,
               W1, b1, We1, be1, root1,
               Wmu, bmu, Wemu, bemu, rootmu,
               Wls, bls, Wels, bels, rootls):
    x = np.asarray(x, np.float32)
    row = np.asarray(edge_index[0], np.int64)
    col = np.asarray(edge_index[1], np.int64)
    ea = np.asarray(edge_attr, np.float32)

    deg = (np.bincount(row, minlength=N_NODES) + 1.0).astype(np.float32)
    dinv = deg ** -0.5
    rdeg = (1.0 / deg).astype(np.float32)
    srw_all = dinv[row] * dinv[col]

    core_of = row // SHARD
    is_hi = (col >= HALF)

    cnt = np.zeros((N_CORES, 2), np.int64)
    for c in range(N_CORES):
        m = core_of == c
        cnt[c, 0] = (~is_hi[m]).sum()
        cnt[c, 1] = is_hi[m].sum()
    UL = int(-(-(cnt[:, 0].max() + 16) // USZ))
    UH = int(-(-(cnt[:, 1].max() + 16) // USZ))
    U = UL + UH
    NCH = U * UC

    we1aug = np.concatenate([np.asarray(We1, np.float32),
                             (np.asarray(be1) + np.asarray(b1))[None, :]], 0)
    wcat = np.concatenate([np.asarray(Wmu), np.asarray(Wls)], 1).astype(np.float32)
    we2aug = np.concatenate([
        np.concatenate([np.asarray(Wemu), np.asarray(Wels)], 1),
        np.concatenate([np.asarray(bemu) + np.asarray(bmu),
                        np.asarray(bels) + np.asarray(bls)])[None, :]], 0)
    bias1r = np.tile((np.asarray(b1) + np.asarray(root1))[None, :], (P, 1)).astype(np.float32)
    bias2r = np.tile(np.concatenate([np.asarray(bmu) + np.asarray(rootmu),
                                     np.asarray(bls) + np.asarray(rootls)])[None, :],
                     (P, 1)).astype(np.float32)
    ident = np.eye(P, dtype=np.float32)

    shared = dict(W1=_bf16(W1), we1aug=_bf16(we1aug), wcat=_bf16(wcat),
                  we2aug=_bf16(we2aug), ident=_bf16(ident),
                  bias1r=bias1r, bias2r=bias2r)

    per_core = []
    for c in range(N_CORES):
        m = core_of == c
        r_loc = (row[m] - c * SHARD).astype(np.int64)
        d = col[m]
        a = ea[m]
        s = srw_all[m]
        hi = is_hi[m]

        gidx_flat = np.zeros(NCH * P, np.int16)
        sidx_flat = np.full(NCH * P, -1, np.int16)
        srw = np.zeros((P, NCH), np.float32)
        at = np.zeros((8, NCH * P), np.float32)

        for h in range(2):
            sel = np.where(hi == (h == 1))[0]
            d_rel = (d[sel] - h * HALF).astype(np.int64)
            Uh = UL if h == 0 else UH
            unit = _assign_units(d_rel, Uh)
            ucnt = np.bincount(unit, minlength=Uh)
            assert ucnt.max() <= USZ, (c, h, ucnt.max())
            order = np.argsort(unit, kind="stable")
            ustart = np.concatenate([[0], np.cumsum(ucnt)])[:-1]
            pos = np.arange(len(sel)) - ustart[unit[order]]
            u_of = unit[order] + (0 if h == 0 else UL)
            e_of = sel[order]
            slot = u_of * USZ + pos
            gidx_flat[slot] = r_loc[e_of].astype(np.int16)
            sidx_flat[slot] = d_rel[order].astype(np.int16)
            srw[slot % P, slot // P] = s[e_of]
            at[:7, slot] = a[e_of].T
            at[7, slot] = 1.0
            # pads -> per-unit safe row (absent from that call's real rows)
            for uu in range(Uh):
                gs = (uu + (0 if h == 0 else UL)) * USZ
                seg = sidx_flat[gs:gs + USZ]
                used = seg[seg >= 0]
                safe = 0
                if used.size:
                    uq = np.unique(used)
                    holes = np.setdiff1d(np.arange(len(uq) + 1), uq,
                                         assume_unique=True)
                    safe = int(holes[0]) if holes.size else 0
                seg[seg < 0] = safe

        def wrap(flat):
            a16 = np.zeros((16, NCH * P // 16), np.int16)
            idx = np.arange(NCH * P)
            a16[idx % 16, idx // 16] = flat
            return np.tile(a16, (8, 1))

        rdegc = np.zeros((P, NB), np.float32)
        ids = np.arange(SHARD)
        rdegc[ids % P, ids // P] = rdeg[c * SHARD + ids]

        xT = np.ascontiguousarray(x[c * SHARD:(c + 1) * SHARD].T)

        dd = dict(gidx=wrap(gidx_flat), sidx=wrap(sidx_flat), srw=srw,
                  at=_bf16(at), xT=_bf16(xT), rdegc=rdegc)
        dd.update(shared)
        per_core.append(dd)
    return per_core, UL, UH


def _build_nc(UL, UH, phases=("tab", "l1", "rs1", "h", "l2", "rs2", "out"),
              ablate=()):
    from concourse import bass, bacc, mybir
    import concourse.tile as tile

    f32 = mybir.dt.float32
    bf16 = mybir.dt.bfloat16
    i16 = mybir.dt.int16
    Relu = mybir.ActivationFunctionType.Relu
    Alu = mybir.AluOpType
    U = UL + UH
    NCH = U * UC

    nc = bacc.Bacc(None, num_devices=N_CORES)

    xT_d = nc.declare_dram_parameter("xT", [P, SHARD], bf16, isOutput=False)
    W1_d = nc.declare_dram_parameter("W1", [IN_F, HID], bf16, isOutput=False)
    we1_d = nc.declare_dram_parameter("we1aug", [8, HID], bf16, isOutput=False)
    wcat_d = nc.declare_dram_parameter("wcat", [HID, P], bf16, isOutput=False)
    we2_d = nc.declare_dram_parameter("we2aug", [8, P], bf16, isOutput=False)
    ident_d = nc.declare_dram_parameter("ident", [P, P], bf16, isOutput=False)
    bias1_d = nc.declare_dram_parameter("bias1r", [P, HID], f32, isOutput=False)
    bias2_d = nc.declare_dram_parameter("bias2r", [P, P], f32, isOutput=False)
    rdegc_d = nc.declare_dram_parameter("rdegc", [P, NB], f32, isOutput=False)
    gidx_d = nc.declare_dram_parameter("gidx", [P, NCH * 8], i16, isOutput=False)
    sidx_d = nc.declare_dram_parameter("sidx", [P, NCH * 8], i16, isOutput=False)
    srw_d = nc.declare_dram_parameter("srw", [P, NCH], f32, isOutput=False)
    at_d = nc.declare_dram_parameter("at", [8, NCH * P], bf16, isOutput=False)
    out_d = nc.declare_dram_parameter("out", [SHARD, P], f32, isOutput=True)

    h1 = nc.dram_tensor("h1", [SHARD, HID], bf16)
    t2 = nc.dram_tensor("t2", [SHARD, P], bf16)
    part1 = nc.dram_tensor("part1", [N_NODES, P], bf16)
    part2 = nc.dram_tensor("part2", [N_NODES, P], bf16)
    rs1 = nc.dram_tensor("rs1", [SHARD, P], bf16)
    rs2 = nc.dram_tensor("rs2", [SHARD, P], bf16)

    with tile.TileContext(nc) as tc:
        with (
            tc.tile_pool(name="const", bufs=1) as cpool,
            tc.tile_pool(name="selfb", bufs=1) as spool,
            tc.tile_pool(name="at", bufs=2) as atpool,
            tc.tile_pool(name="g", bufs=3) as gpool,
            tc.tile_pool(name="m", bufs=3) as mpool,
            tc.tile_pool(name="work", bufs=3) as wpool,
            tc.tile_pool(name="pse", bufs=4, space="PSUM") as pse,
            tc.tile_pool(name="psn", bufs=2, space="PSUM") as psn,
        ):
            xT_t = cpool.tile([P, SHARD], bf16)
            W1_t = cpool.tile([IN_F, HID], bf16)
            we1_t = cpool.tile([8, HID], bf16)
            wcat_t = cpool.tile([HID, P], bf16)
            we2_t = cpool.tile([8, P], bf16)
            ident_t = cpool.tile([P, P], bf16)
            bias1_t = cpool.tile([P, HID], f32)
            bias2_t = cpool.tile([P, P], f32)
            rdegc_t = cpool.tile([P, NB], f32)
            gidx_t = cpool.tile([P, NCH * 8], i16)
            sidx_t = cpool.tile([P, NCH * 8], i16)
            srw_t = cpool.tile([P, NCH], f32)
            for t, d in ((xT_t, xT_d), (W1_t, W1_d), (we1_t, we1_d),
                         (wcat_t, wcat_d), (we2_t, we2_d), (ident_t, ident_d),
                         (bias1_t, bias1_d), (bias2_t, bias2_d),
                         (rdegc_t, rdegc_d), (gidx_t, gidx_d),
                         (sidx_t, sidx_d), (srw_t, srw_d)):
                nc.sync.dma_start(out=t[:], in_=d[:])

            # zero both partial tables (overlaps tab / L1)
            zt = cpool.tile([P, SHARD // 2], bf16)
            nc.vector.memset(zt[:], 0.0)
            for tgt in (part1, part2):
                for r0 in range(0, N_NODES, SHARD // 2):
                    nc.sync.dma_start(out=tgt[r0:r0 + SHARD // 2, :], in_=zt[:])

            selfbuf1 = [spool.tile([P, HID], bf16, name=f"s1_{b}", tag=f"s1_{b}")
                        for b in range(NB)]
            selfbuf2 = [spool.tile([P, P], bf16, name=f"s2_{b}", tag=f"s2_{b}")
                        for b in range(NB)]

            # ---- tab: h1 = x_own @ W1 ; selfbuf1 = relu(h1 + b1 + root1) ----
            for j in range(NB if "tab" in phases else 0):
                lo = j * P
                nj = min(P, SHARD - lo)
                ps = psn.tile([P, HID], f32, tag="pt")
                nc.tensor.matmul(out=ps[:nj, :], lhsT=xT_t[:, lo:lo + nj],
                                 rhs=W1_t[:], start=True, stop=True)
                h1b = wpool.tile([P, HID], bf16, tag="h1b")
                nc.vector.tensor_copy(out=h1b[:nj, :], in_=ps[:nj, :])
                nc.sync.dma_start(out=h1[lo:lo + nj, :], in_=h1b[:nj, :])
                pre = wpool.tile([P, HID], f32, tag="pre1")
                nc.vector.tensor_tensor(out=pre[:], in0=ps[:], in1=bias1_t[:],
                                        op=Alu.add)
                nc.scalar.activation(selfbuf1[j][:], pre[:], Relu)

            # ---- edge pass (units of 1024 edges) ----
            def edge_pass(table, we_t, part, layer):
                for u in range(U):
                    if u % ATB == 0:
                        nu = min(ATB, U - u)
                        at_t = atpool.tile([8, ATB * USZ], bf16, tag="at")
                        nc.sync.dma_start(
                            out=at_t[:, :nu * USZ],
                            in_=at_d[:, u * USZ:(u + nu) * USZ])
                    gt = gpool.tile([P, UC, P], bf16, tag="gt")
                    if "nogather" not in ablate:
                        nc.gpsimd.dma_gather(
                            gt[:], table[:],
                            gidx_t[:, u * (USZ // 16):(u + 1) * (USZ // 16)],
                            USZ, USZ, P)
                    msg = mpool.tile([P, UC, P], bf16, tag="msg")
                    for q in range(UC):
                        cidx = u * UC + q
                        a0 = (u % ATB) * USZ + q * P
                        qq = q % 4
                        if "noeps" in ablate:
                            if q == 0:
                                nc.vector.tensor_copy(out=msg[:], in_=gt[:])
                            continue
                        if qq == 0:
                            eps = pse.tile([P, 4, P], f32, tag="eps")
                        nc.tensor.matmul(out=eps[:, qq, :],
                                         lhsT=at_t[:, a0:a0 + P],
                                         rhs=we_t[:], start=True, stop=False)
                        nc.tensor.matmul(out=eps[:, qq, :], lhsT=ident_t[:],
                                         rhs=gt[:, q, :], start=False, stop=True)
                        if cidx % 2 == 0:
                            nc.vector.tensor_scalar(
                                out=msg[:, q, :], in0=eps[:, qq, :],
                                scalar1=0.0, scalar2=srw_t[:, cidx:cidx + 1],
                                op0=Alu.max, op1=Alu.mult)
                        else:
                            nc.scalar.activation(msg[:, q, :], eps[:, qq, :],
                                                 Relu,
                                                 scale=srw_t[:, cidx:cidx + 1])
                    if "noscat" in ablate:
                        continue
                    off = 0 if u < UL else HALF
                    nc.gpsimd.dma_scatter_add(
                        part[off:off + HALF, :], msg[:],
                        sidx_t[:, u * (USZ // 16):(u + 1) * (USZ // 16)],
                        USZ, USZ, P)

            if "l1" in phases:
                edge_pass(h1, we1_t, part1, 1)

            if "rs1" in phases:
                nc.gpsimd.collective_compute(
                    "ReduceScatter", Alu.add,
                    replica_groups=[list(range(N_CORES))],
                    ins=[part1[:]], outs=[rs1[:]])

            # ---- h = relu(agg1 + self1/deg); t2 = h @ wcat; selfbuf2 ----
            for j in range(NB if "h" in phases else 0):
                lo = j * P
                nj = min(P, SHARD - lo)
                zb = wpool.tile([P, P], bf16, tag="zb")
                nc.sync.dma_start(out=zb[:nj, :], in_=rs1[lo:lo + nj, :])
                v = wpool.tile([P, HID], f32, tag="v")
                nc.vector.tensor_scalar(out=v[:], in0=selfbuf1[j][:],
                                        scalar1=rdegc_t[:, j:j + 1], scalar2=None,
                                        op0=Alu.mult)
                w = wpool.tile([P, HID], f32, tag="w")
                nc.vector.tensor_tensor(out=w[:], in0=zb[:], in1=v[:], op=Alu.add)
                hb = wpool.tile([P, HID], bf16, tag="hb")
                nc.scalar.activation(hb[:], w[:], Relu)
                pst = psn.tile([P, P], bf16, tag="pT")
                nc.tensor.transpose(out=pst[:], in_=hb[:], identity=ident_t[:])
                hT = wpool.tile([P, P], bf16, tag="hT")
                nc.vector.tensor_copy(out=hT[:], in_=pst[:])
                t2ps = psn.tile([P, P], f32, tag="pt")
                nc.tensor.matmul(out=t2ps[:], lhsT=hT[:], rhs=wcat_t[:],
                                 start=True, stop=True)
                t2b = wpool.tile([P, P], bf16, tag="t2b")
                nc.vector.tensor_copy(out=t2b[:nj, :], in_=t2ps[:nj, :])
                nc.sync.dma_start(out=t2[lo:lo + nj, :], in_=t2b[:nj, :])
                pre2 = wpool.tile([P, P], f32, tag="pre2")
                nc.vector.tensor_tensor(out=pre2[:], in0=t2ps[:], in1=bias2_t[:],
                                        op=Alu.add)
                nc.scalar.activation(selfbuf2[j][:], pre2[:], Relu)

            if "l2" in phases:
                edge_pass(t2, we2_t, part2, 2)

            if "rs2" in phases:
                nc.gpsimd.collective_compute(
                    "ReduceScatter", Alu.add,
                    replica_groups=[list(range(N_CORES))],
                    ins=[part2[:]], outs=[rs2[:]])

            # ---- out = agg2 + self2/deg ----
            if "out" not in phases:
                zo = wpool.tile([P, P], f32, tag="zo")
                nc.vector.memset(zo[:], 0.0)
                nc.sync.dma_start(out=out_d[0:P, :], in_=zo[:])
            for j in range(NB if "out" in phases else 0):
                lo = j * P
                nj = min(P, SHARD - lo)
                zb = wpool.tile([P, P], bf16, tag="zb2")
                nc.sync.dma_start(out=zb[:nj, :], in_=rs2[lo:lo + nj, :])
                v = wpool.tile([P, P], f32, tag="v2")
                nc.vector.tensor_scalar(out=v[:], in0=selfbuf2[j][:],
                                        scalar1=rdegc_t[:, j:j + 1], scalar2=None,
                                        op0=Alu.mult)
                o = wpool.tile([P, P], f32, tag="o")
                nc.vector.tensor_tensor(out=o[:], in0=zb[:], in1=v[:], op=Alu.add)
                nc.sync.dma_start(out=out_d[lo:lo + nj, :], in_=o[:nj, :])

    nc.finalize()
    return nc


_CACHE = {}


def kernel(**inputs):
    from concourse.bass_utils import run_bass_kernel_spmd

    per_core, UL, UH = _host_prep(**inputs)
    key = (UL, UH)
    if key not in _CACHE:
        _CACHE[key] = _build_nc(UL, UH)
    nc = _CACHE[key]
    r = None
    for attempt in range(3):
        try:
            r = run_bass_kernel_spmd(nc, per_core, list(range(N_CORES)))
            break
        except Exception:
            if attempt == 2:
                raise
            import time as _time
            _time.sleep(5.0)
    outs = [r.results[c]["out"] for c in range(N_CORES)]
    full = np.concatenate(outs, axis=0)
    mu = np.ascontiguousarray(full[:, :OUT_F])
    logstd = np.ascontiguousarray(full[:, OUT_F:])
    return (mu, logstd)
